# revision 1
# baseline (speedup 1.0000x reference)
"""Trainium2 Bass kernel for CuGraphRelGraphConv (basis-decomposed relational
graph conv) on 8 NeuronCores.

Math (reference):
    msg_e   = coeff[etype_e] (x) feat[src_e]            # [E, 2, 64]
    agg     = segment_sum(msg, dst)                     # [N, 2, 64]
    h       = agg.reshape(N,128) @ W.reshape(128,64) + bias + feat @ loop_w

Device mapping per core (dst-sharded, 12544 nodes/core, 196 windows x 64):
  - gather: per-edge rows of a bf16 table [N, 128] ([feat | 0] padding to
    256B) via gpsimd.dma_gather, int16 indices bucketed into 4 banks of
    2^15 rows (src = bank*32768 + idx16).
  - scaled selection: S01[e, (b, slot)] = coeff[et_e, b] * (dstl_e == slot)
    built on DVE from host-precomputed per-edge (dstl, c0, c1).
  - scatter matmul per 128-edge tile: PSUM agg[d, (b,slot)] += G_t^T @ S01_t
  - per window: h[slot, o] = sum_b agg_b^T-free matmuls with W[b] + fused
    self-loop + bias via an extra ones-row in featT.
Edges, windows and bank buckets are bucketed/padded on host; the static
schedule (slot counts) is the max over the 8 cores so one SPMD program
serves all cores.
"""
import sys

sys.path.insert(0, "/opt/trn_rl_repo")

import numpy as np
import ml_dtypes

import concourse.bass as bass
import concourse.bacc as bacc
import concourse.mybir as mybir
from concourse.bass_utils import run_bass_kernel_spmd
from concourse.tile import TileContext

BF16 = ml_dtypes.bfloat16

# ---------------- problem constants (full size, hardcoded) ----------------
N_NODES = 100000
N_EDGES = 1600000
IN_FEAT = 64
OUT_FEAT = 64
NUM_RELS = 8
NUM_BASES = 2
N_CORES = 8

WIN = 64              # dst nodes per window (= PSUM slot count)
BANK_BITS = 15        # int16 gather index reach


class Config:
    def __init__(self, n_nodes, n_cores=8, win=64, gw=12, sw=4,
                 bank_bits=BANK_BITS):
        assert gw % sw == 0, "S01 groups must nest inside gather groups"
        self.n_nodes = n_nodes
        self.bank_bits = bank_bits
        self.bank = 1 << bank_bits
        self.n_cores = n_cores
        self.win = win
        self.gw = gw                      # windows per gather group
        self.sw = sw                      # windows per S01-build group
        nw = -(-n_nodes // (n_cores * win))   # windows per core
        # round windows per core up to a multiple of gw
        nw = -(-nw // gw) * gw
        self.nw = nw
        self.npc = nw * win               # nodes per core (padded)
        self.nbank = -(-n_nodes // self.bank)
        self.ngroup = nw // gw


def make_schedule(cfg, src, dst, etypes, coeff):
    """Bucket edges by (core, window, bank); build per-core padded slot
    arrays in two orders:
      - gather order  : (group, bank, window)  -> int16 idx arrays
      - window order  : (window, bank)         -> dstl / c0 / c1 arrays
    Slot counts are shared across cores (max), so the SPMD program is
    uniform. Returns (sched_dict, per_core_input_arrays)."""
    K, NW, NB, W = cfg.n_cores, cfg.nw, cfg.nbank, cfg.win
    core = dst // cfg.npc
    w = (dst - core * cfg.npc) // W
    dstl = dst % W
    bank = src >> cfg.bank_bits
    idx16 = (src & (cfg.bank - 1)).astype(np.int16)
    cc = coeff[etypes].astype(np.float32)            # [E, 2]

    counts = np.zeros((K, NW, NB), np.int64)
    np.add.at(counts, (core, w, bank), 1)
    slots = 128 * np.ceil(counts.max(axis=0) / 128).astype(np.int64)  # [NW, NB]
    nt = slots // 128                                 # tiles per (w, b)
    tiles_w = nt.sum(axis=1)                          # tiles per window
    n_tiles = int(tiles_w.sum())
    n_slots = n_tiles * 128

    # window-major slot offset of bucket (w, b)
    woff = np.zeros((NW, NB), np.int64)
    flat = slots.reshape(-1)
    woff.reshape(-1)[1:] = np.cumsum(flat)[:-1]
    # gather-order slot offset of bucket (w, b): order (g, b, w in g)
    goff = np.zeros((NW, NB), np.int64)
    run = 0
    gchunk = np.zeros((NW, NB), np.int64)   # chunk offset within group tile
    for g in range(cfg.ngroup):
        grun = 0
        for b in range(NB):
            for wi in range(g * cfg.gw, (g + 1) * cfg.gw):
                goff[wi, b] = run
                gchunk[wi, b] = grun
                run += slots[wi, b]
                grun += slots[wi, b] // 128
    assert run == n_slots

    group_nidx = np.zeros((cfg.ngroup, NB), np.int64)
    for g in range(cfg.ngroup):
        for b in range(NB):
            group_nidx[g, b] = slots[g * cfg.gw:(g + 1) * cfg.gw, b].sum()
    group_chunks = group_nidx.sum(axis=1) // 128      # C_g per group

    # ---- per-core arrays ----
    per_core = []
    for k in range(K):
        m = core == k
        ew, eb = w[m], bank[m]
        edstl, eidx, ecc = dstl[m], idx16[m], cc[m]
        # order edges by (w, b) then place sequentially into both layouts
        order = np.lexsort((eb, ew))
        ew, eb, edstl, eidx, ecc = (
            ew[order], eb[order], edstl[order], eidx[order], ecc[order])
        # position of each edge within its bucket
        # edges are sorted by (w,b); per-bucket running index:
        bucket_id = ew * NB + eb
        # stable running counter per bucket
        pos = np.zeros(len(ew), np.int64)
        if len(ew):
            change = np.r_[True, bucket_id[1:] != bucket_id[:-1]]
            start_of_run = np.flatnonzero(change)
            run_id = np.cumsum(change) - 1
            pos = np.arange(len(ew)) - start_of_run[run_id]

        g_idx = np.zeros(n_slots, np.int16)           # gather order, pad idx 0
        wm_dstl = np.full(n_slots, W, np.float32)     # window order, pad -> W
        wm_cc = np.zeros((n_slots, 2), np.float32)

        gslot = goff[ew, eb] + pos
        wslot = woff[ew, eb] + pos
        g_idx[gslot] = eidx
        wm_dstl[wslot] = edstl
        wm_cc[wslot] = ecc

        # wrap gather indices: idx i -> [16, n/16] partition-wrapped, x8
        wrapped = g_idx.reshape(n_slots // 16, 16).T  # [16, n/16]
        wrapped = np.tile(wrapped, (8, 1))            # [128, n/16]

        # window-major per-slot metadata -> [128, n_tiles] layout
        # slot s = tile*128 + p  ->  partition p, column tile
        dstl_t = wm_dstl.reshape(n_tiles, 128).T.astype(BF16)      # [128, T]
        cc_t = np.ascontiguousarray(
            wm_cc.reshape(n_tiles, 128, 2).transpose(1, 0, 2)
        ).reshape(128, n_tiles * 2).astype(BF16)                   # [128, 2T]
        per_core.append({"idx": wrapped, "dstl": dstl_t, "cc": cc_t})

    sched = {
        "slots": slots, "nt": nt, "tiles_w": tiles_w, "n_tiles": n_tiles,
        "n_slots": n_slots, "woff": woff, "gchunk": gchunk,
        "group_nidx": group_nidx, "group_chunks": group_chunks,
    }
    return sched, per_core


def build_program(cfg, sched):
    import os
    dbg_stage = int(os.environ.get("K_STAGE", "9"))
    NW, NB, W = cfg.nw, cfg.nbank, cfg.win
    n_tiles, n_slots = sched["n_tiles"], sched["n_slots"]
    nt, tiles_w, woff = sched["nt"], sched["tiles_w"], sched["woff"]
    gchunk = sched["gchunk"]
    group_nidx, group_chunks = sched["group_nidx"], sched["group_chunks"]
    BANK = cfg.bank
    bankrows = [min(BANK, cfg.n_nodes - b * BANK) for b in range(NB)]

    nc = bacc.Bacc("TRN2", target_bir_lowering=False, debug=False,
                   num_devices=cfg.n_cores, num_swdge_queues=4)
    dt = mybir.dt

    table = nc.dram_tensor("table", [cfg.n_nodes, 128], dt.bfloat16,
                           kind="ExternalInput").ap()
    idx_d = nc.dram_tensor("idx", [128, n_slots // 16], dt.int16,
                           kind="ExternalInput").ap()
    dstl_d = nc.dram_tensor("dstl", [128, n_tiles], dt.bfloat16,
                            kind="ExternalInput").ap()
    cc_d = nc.dram_tensor("cc", [128, 2 * n_tiles], dt.bfloat16,
                          kind="ExternalInput").ap()
    featT_d = nc.dram_tensor("featT", [65, cfg.npc], dt.float32,
                             kind="ExternalInput").ap()
    wmat_d = nc.dram_tensor("wmat", [64, 2 * 64], dt.bfloat16,
                            kind="ExternalInput").ap()   # [d, (b,o)]
    lw_d = nc.dram_tensor("lw65", [65, 64], dt.float32,
                          kind="ExternalInput").ap()
    iota_d = nc.dram_tensor("iota", [128, W], dt.bfloat16,
                            kind="ExternalInput").ap()
    out_d = nc.dram_tensor("out", [cfg.npc, 64], dt.float32,
                           kind="ExternalOutput").ap()

    max_cg = int(group_chunks.max())
    max_gidx = int(group_nidx.sum(axis=1).max())
    sw_tiles = [int(tiles_w[s * cfg.sw:(s + 1) * cfg.sw].sum())
                for s in range(NW // cfg.sw)]
    max_st = max(sw_tiles) if sw_tiles else 0

    with TileContext(nc) as tc:
        with (
            tc.tile_pool(name="const", bufs=1) as cpool,
            tc.tile_pool(name="gather", bufs=2) as gpool,
            tc.tile_pool(name="gidx", bufs=2) as ipool,
            tc.tile_pool(name="sel", bufs=2) as spool,
            tc.tile_pool(name="acopy", bufs=3) as apool,
            tc.tile_pool(name="hout", bufs=3) as hpool,
            tc.tile_pool(name="psum_a", bufs=2, space="PSUM") as pa,
            tc.tile_pool(name="psum_h", bufs=2, space="PSUM") as ph,
        ):
            # resident constants / metadata
            dstl_t = cpool.tile([128, n_tiles], dt.bfloat16)
            nc.sync.dma_start(out=dstl_t[:], in_=dstl_d[:])
            cc_t = cpool.tile([128, 2 * n_tiles], dt.bfloat16)
            nc.sync.dma_start(out=cc_t[:], in_=cc_d[:])
            featT_t = cpool.tile([65, cfg.npc], dt.float32)
            nc.sync.dma_start(out=featT_t[:], in_=featT_d[:])
            wmat_t = cpool.tile([64, 2 * 64], dt.bfloat16)
            nc.sync.dma_start(out=wmat_t[:], in_=wmat_d[:])
            lw_t = cpool.tile([65, 64], dt.float32)
            nc.sync.dma_start(out=lw_t[:], in_=lw_d[:])
            iota_t = cpool.tile([128, W], dt.bfloat16)
            nc.sync.dma_start(out=iota_t[:], in_=iota_d[:])

            for g in range(cfg.ngroup if dbg_stage >= 1 else 0):
                cg = int(group_chunks[g])
                gt = gpool.tile([128, max_cg, 128], dt.bfloat16, tag="g")
                nidx_g = int(group_nidx[g].sum())
                it = ipool.tile([128, max_gidx // 16], dt.int16, tag="i")
                idx_off = int(group_nidx[:g].sum()) // 16
                nc.sync.dma_start(
                    out=it[:, : nidx_g // 16],
                    in_=idx_d[:, idx_off: idx_off + nidx_g // 16])
                # gather calls per bank, capped at MAX_GATHER idx per call
                MAX_GATHER = 8192
                coff = 0
                ioff = 0
                for b in range(NB):
                    nidx = int(group_nidx[g, b])
                    done = 0
                    while done < nidx:
                        n1 = min(MAX_GATHER, nidx - done)
                        nchunk = n1 // 128
                        nc.gpsimd.dma_gather(
                            out_ap=gt[:, coff: coff + nchunk, :],
                            in_ap=table[b * BANK: b * BANK + bankrows[b], :],
                            idxs_ap=it[:, ioff: ioff + n1 // 16],
                            num_idxs=n1,
                            num_idxs_reg=n1,
                            elem_size=128,
                            queue_num=b % 4,
                            single_packet=False,
                        )
                        coff += nchunk
                        ioff += n1 // 16
                        done += n1

                if dbg_stage < 2:
                    continue
                # process this group's windows
                for s in range(g * cfg.gw // cfg.sw,
                               (g + 1) * cfg.gw // cfg.sw):
                    w0 = s * cfg.sw
                    t0 = int(tiles_w[:w0].sum())
                    ts = int(tiles_w[w0: w0 + cfg.sw].sum())
                    if ts > 0:
                        onehot = spool.tile([128, max_st, W], dt.bfloat16,
                                            tag="oh")
                        s01 = spool.tile([128, max_st, 2, W], dt.bfloat16,
                                         tag="s01")
                        nc.vector.tensor_tensor(
                            out=onehot[:, :ts, :],
                            in0=dstl_t[:, t0: t0 + ts].unsqueeze(-1)
                                .to_broadcast([128, ts, W]),
                            in1=iota_t[:].unsqueeze(1)
                                .to_broadcast([128, ts, W]),
                            op=mybir.AluOpType.is_equal,
                        )
                        nc.vector.tensor_tensor(
                            out=s01[:, :ts, :, :],
                            in0=onehot[:, :ts, :].unsqueeze(2)
                                .to_broadcast([128, ts, 2, W]),
                            in1=cc_t[:, 2 * t0: 2 * (t0 + ts)]
                                .rearrange("p (t c) -> p t c", c=2)
                                .unsqueeze(-1).to_broadcast([128, ts, 2, W]),
                            op=mybir.AluOpType.mult,
                        )
                    if dbg_stage < 3:
                        continue
                    for wi in range(w0, w0 + cfg.sw):
                        tw = int(tiles_w[wi])
                        hps = ph.tile([64, 64], dt.float32, tag="h")
                        if tw > 0:
                            aps = pa.tile([64, 2 * W], dt.float32, tag="a")
                            ti = 0
                            for b in range(NB):
                                for j in range(int(nt[wi, b])):
                                    # window-major tile position rel. sgroup
                                    st_tile = int(woff[wi, b]) // 128 + j - t0
                                    rhs = s01[:, st_tile, :, :]\
                                        .rearrange("p c w -> p (c w)")
                                    nc.tensor.matmul(
                                        out=aps[:],
                                        lhsT=gt[:, int(gchunk[wi, b]) + j,
                                                0:64],
                                        rhs=rhs,
                                        start=(ti == 0),
                                        stop=(ti == tw - 1),
                                    )
                                    ti += 1
                            # agg [64d, (b,slot)] fp32 -> bf16 SBUF
                            aggs = apool.tile([64, 2 * W], dt.bfloat16,
                                              tag="agg")
                            nc.scalar.activation(
                                out=aggs[:], in_=aps[:],
                                func=mybir.ActivationFunctionType.Copy)
                        # h = sum_b agg_b^T(free) @ W_b  (+ selfloop w/ bias)
                        if tw > 0:
                            for b2 in range(2):
                                nc.tensor.matmul(
                                    out=hps[:],
                                    lhsT=aggs[:, b2 * W:(b2 + 1) * W],
                                    rhs=wmat_t[:, b2 * 64:(b2 + 1) * 64],
                                    start=(b2 == 0),
                                    stop=False,
                                )
                        nc.tensor.matmul(
                            out=hps[:],
                            lhsT=featT_t[:, wi * W: wi * W + 64],
                            rhs=lw_t[:],
                            start=(tw == 0),
                            stop=True,
                        )
                        hs = hpool.tile([64, 64], dt.float32, tag="hs")
                        nc.scalar.activation(
                            out=hs[:], in_=hps[:],
                            func=mybir.ActivationFunctionType.Copy)
                        nc.sync.dma_start(
                            out=out_d[wi * W: wi * W + 64, :], in_=hs[:])

    nc.compile()
    return nc


def make_inputs(cfg, per_core_sched, feat, W, coeff, h_bias, loop_weight):
    """Host-side tensor prep shared across cores + per-core metadata."""
    n = cfg.n_nodes
    table = np.zeros((n, 128), BF16)
    table[:, 0:64] = feat.astype(BF16)

    featT = np.zeros((65, cfg.npc), np.float32)
    featT[64, :] = 1.0
    ncore_nodes = min(cfg.npc, 10**18)
    # filled per core below
    wmat = np.ascontiguousarray(
        W.transpose(1, 0, 2).reshape(64, 2 * 64)).astype(BF16)  # [d,(b,o)]
    lw65 = np.concatenate(
        [loop_weight.astype(np.float32), h_bias[None].astype(np.float32)], 0)
    iota = np.tile(np.arange(cfg.win, dtype=np.float32)[None], (128, 1))\
        .astype(BF16)

    in_maps = []
    for k in range(cfg.n_cores):
        fT = featT.copy()
        lo = k * cfg.npc
        hi = min((k + 1) * cfg.npc, n)
        if hi > lo:
            fT[0:64, : hi - lo] = feat[lo:hi].T
        pc = per_core_sched[k]
        in_maps.append({
            "table": table,
            "idx": pc["idx"],
            "dstl": pc["dstl"],
            "cc": pc["cc"],
            "featT": fT,
            "wmat": wmat,
            "lw65": lw65,
            "iota": iota,
        })
    return in_maps


def run(cfg, feat, W, coeff, h_bias, loop_weight, src, dst, etypes,
        trace=False, sim=False):
    sched, per_core = make_schedule(
        cfg, src.astype(np.int64), dst.astype(np.int64),
        etypes.astype(np.int64), np.asarray(coeff, np.float32))
    nc = build_program(cfg, sched)
    in_maps = make_inputs(cfg, per_core, np.asarray(feat, np.float32),
                          np.asarray(W, np.float32),
                          np.asarray(coeff, np.float32),
                          np.asarray(h_bias, np.float32),
                          np.asarray(loop_weight, np.float32))
    if sim:
        import concourse.bass_interp as bass_interp
        msim = bass_interp.MultiCoreSim(nc, cfg.n_cores)
        for k in range(cfg.n_cores):
            for name, arr in in_maps[k].items():
                msim.cores[k].tensor(name)[:] = arr
        msim.simulate()
        outs = [np.array(msim.cores[k].tensor("out"))
                for k in range(cfg.n_cores)]
        h = np.concatenate(outs, axis=0)[: cfg.n_nodes]
        return h, None
    res = run_bass_kernel_spmd(nc, in_maps, list(range(cfg.n_cores)),
                               trace=trace)
    outs = [res.results[k]["out"] for k in range(cfg.n_cores)]
    h = np.concatenate(outs, axis=0)[: cfg.n_nodes]
    return h, res


def kernel(feat, W, coeff, h_bias, loop_weight, src, dst, etypes):
    cfg = Config(N_NODES)
    h, _ = run(cfg, feat, W, coeff, h_bias, loop_weight, src, dst, etypes)
    return h.astype(np.float32)



# revision 2
# speedup vs baseline: 1.0141x; 1.0141x over previous
"""Trainium2 Bass kernel v2 for CuGraphRelGraphConv on 8 NeuronCores.

Dense-gather design (see sched2.py for the host schedule):
  - dst nodes snake-dealt by degree to 8 cores x 196 windows of 64 slots
    -> near-equal per-window edge counts across cores (SPMD-uniform).
  - per core: 14 groups x 14 windows; slot stream (group, bank, window, src)
    dense, segment sizes padded to cross-core max (128-aligned, pad idx 0).
  - gather: per-(g,b) segments split into <=2048-idx sub-calls round-robin
    over all 4 SWDGE queues -> ~1.5-2.0 ns/idx sustained.
  - scatter: per "appearance" (tile x window) one 128-col matmul
    G_chunk^T @ S01 accumulating into a 4-window PSUM pack [64, 512];
    S01 built on DVE from per-appearance dstl/cc metadata (sentinel 64
    for foreign slots of straddle tiles).
  - h = sum_b aggT_b @ W_b + [feat|1] @ [loop_w; bias] per window,
    copied out via per-pack DMA.
"""
import sys

sys.path.insert(0, "/opt/trn_rl_repo")

import numpy as np
import ml_dtypes

import concourse.bass as bass
import concourse.bacc as bacc
import concourse.mybir as mybir
from concourse.bass_utils import run_bass_kernel_spmd
from concourse.tile import TileContext

import numpy as np
N_NODES = 100000
N_EDGES = 1600000
K = 8
WIN = 32
NG = 14           # groups per core
_wins_per_core = -(-(-(-N_NODES // WIN) // K))
GW = -(-_wins_per_core // NG)   # windows per group
NW = NG * GW      # windows per core
NB = 4            # src banks (int16 gather reach)
BANK = 32768
NPC = NW * WIN
SUBCALL = 1920   # 121 descriptors < 128-deep SWDGE ring
SENT = float(WIN)  # dstl sentinel


def assign_nodes(dst):
    """Snake-deal nodes by degree -> (core, window, slot) + flat maps."""
    deg = np.bincount(dst, minlength=N_NODES)
    order = np.argsort(-deg, kind="stable")
    nwt = K * NW
    winf = np.empty(N_NODES, np.int64)
    slot = np.empty(N_NODES, np.int64)
    r = 0
    for off in range(0, N_NODES, nwt):
        ch = order[off: off + nwt]
        cols = np.arange(len(ch))
        if r % 2 == 1:
            cols = nwt - 1 - cols
        winf[ch] = cols
        slot[ch] = r
        r += 1
    assert slot.max() < WIN
    return winf, slot


def make_schedule(src, dst, etypes, coeff):
    src = np.asarray(src, np.int64)
    dst = np.asarray(dst, np.int64)
    etypes = np.asarray(etypes, np.int64)
    coeff = np.asarray(coeff, np.float32)

    winf, slot = assign_nodes(dst)
    core_of, w_of = winf // NW, winf % NW

    ek = core_of[dst]                  # core
    ew = w_of[dst]                     # window in core
    eg = ew // GW                      # group
    ewl = ew % GW                      # window in group
    eb = src >> 15                     # bank
    eidx = (src & (BANK - 1)).astype(np.int64)
    edstl = slot[dst]                  # 0..63
    ecc = coeff[etypes]                # [E, 2]

    # counts per (k, g, b, wl)
    key = ((ek * NG + eg) * NB + eb) * GW + ewl
    C = np.bincount(key, minlength=K * NG * NB * GW) \
        .reshape(K, NG, NB, GW).astype(np.int64)
    S = C.sum(axis=3)                                  # [K, NG, NB]
    Sstar = 128 * np.ceil(S.max(axis=0) / 128).astype(np.int64)  # [NG, NB]

    starts = np.cumsum(C, axis=3) - C                  # excl cumsum [K,NG,NB,GW]
    ends = starts + C
    lo = starts.min(axis=0)                            # [NG, NB, GW]
    hi = ends.max(axis=0)

    seg_off = np.cumsum(Sstar, axis=1) - Sstar         # [NG, NB] within group
    GS = Sstar.sum(axis=1)                             # group sizes
    grp_off = np.cumsum(GS) - GS
    S_total = int(GS.sum())

    # --- edge slot positions ---
    # order edges by (k, g, b, wl, src); per-bucket running position
    perm = np.lexsort((src, ewl, eb, eg, ek))
    k_s, g_s, b_s, wl_s = ek[perm], eg[perm], eb[perm], ewl[perm]
    bucket = ((k_s * NG + g_s) * NB + b_s) * GW + wl_s
    change = np.r_[True, bucket[1:] != bucket[:-1]]
    run_start = np.flatnonzero(change)
    run_id = np.cumsum(change) - 1
    pos = np.arange(len(perm)) - run_start[run_id]
    spos = (grp_off[g_s] + seg_off[g_s, b_s]
            + starts[k_s, g_s, b_s, wl_s] + pos)       # slot within core

    # per-core slot arrays
    idx_stream = np.zeros((K, S_total), np.int16)
    wl_slot = np.full((K, S_total), -1, np.int64)
    dstl_slot = np.full((K, S_total), WIN, np.float32)
    cc_slot = np.zeros((K, S_total, 2), np.float32)
    idx_stream[k_s, spos] = eidx[perm].astype(np.int16)
    wl_slot[k_s, spos] = wl_s
    dstl_slot[k_s, spos] = edstl[perm]
    cc_slot[k_s, spos] = ecc[perm]

    # --- appearances ---
    # per (g, w): ordered list over b, tiles t in seg (g,b) where
    # [lo, hi) of (g,b,w) intersects tile t. Order: (g, w, b, t).
    apps = []            # dicts: g, wl, b, t, chunk (within group), a (index)
    app_ranges = {}      # (g, wl) -> (a0, a1) contiguous? order by (g,w)
    for g in range(NG):
        for wl in range(GW):
            first = len(apps)
            for b in range(NB):
                l, h = lo[g, b, wl], hi[g, b, wl]
                if h <= l:
                    continue
                t0, t1 = l // 128, (h - 1) // 128 + 1
                for t in range(t0, t1):
                    apps.append(dict(
                        g=g, wl=wl, b=b, t=t,
                        chunk=(seg_off[g, b] // 128) + t))
            app_ranges[(g, wl)] = (first, len(apps))
    A = len(apps)

    # appearance metadata arrays [K, 128, A]
    dstl_app = np.full((K, 128, A), SENT, np.float32)
    cc_app = np.zeros((K, 128, A, 2), np.float32)
    for a, ap in enumerate(apps):
        g, wl, b, t = ap["g"], ap["wl"], ap["b"], ap["t"]
        s0 = grp_off[g] + seg_off[g, b] + 128 * t
        sl = slice(s0, s0 + 128)
        m = wl_slot[:, sl] == wl                       # [K, 128]
        dstl_app[:, :, a] = np.where(m, dstl_slot[:, sl], SENT)
        cc_app[:, :, a, :] = cc_slot[:, sl, :]

    # --- gather calls ---
    calls = []   # (g, col0, ncols16, chunk0, nchunks, queue)
    q = 0
    for g in range(NG):
        for b in range(NB):
            sz = int(Sstar[g, b])
            off = 0
            while off < sz:
                nn = min(SUBCALL, sz - off)
                s0 = grp_off[g] + seg_off[g, b] + off
                calls.append(dict(
                    g=g, idx0=int(s0), nidx=int(nn),
                    chunk0=int((seg_off[g, b] + off) // 128),
                    queue=q % 4))
                q += 1
                off += nn

    return dict(
        winf=winf, slot=slot, C=C, Sstar=Sstar, seg_off=seg_off, GS=GS,
        grp_off=grp_off, S_total=S_total, idx_stream=idx_stream,
        apps=apps, app_ranges=app_ranges, A=A,
        dstl_app=dstl_app, cc_app=cc_app, calls=calls,
        wl_slot=wl_slot,
    )



BF16 = ml_dtypes.bfloat16
PACK = 512 // (2 * WIN)       # windows per PSUM pack ([64, 512] fp32 bank)


def build_program(sched):
    import os
    stage = int(os.environ.get("K2_STAGE", "9"))
    dt = mybir.dt
    GS = sched["GS"]
    grp_off = sched["grp_off"]
    S_total = sched["S_total"]
    A = sched["A"]
    apps = sched["apps"]
    app_ranges = sched["app_ranges"]
    calls = sched["calls"]
    seg_off = sched["seg_off"]

    calls_by_g = [[c for c in calls if c["g"] == g] for g in range(NG)]

    nc = bacc.Bacc("TRN2", target_bir_lowering=False, debug=False,
                   num_devices=K, num_swdge_queues=4)

    table_d = nc.dram_tensor("table", [N_NODES, 128], dt.bfloat16,
                             kind="ExternalInput").ap()
    idx_d = nc.dram_tensor("idx", [128, S_total // 16], dt.int16,
                           kind="ExternalInput").ap()
    dstl_d = nc.dram_tensor("dstl", [128, A], dt.bfloat16,
                            kind="ExternalInput").ap()
    cc_d = nc.dram_tensor("cc", [128, 2 * A], dt.bfloat16,
                          kind="ExternalInput").ap()
    featT_d = nc.dram_tensor("featT", [65, NPC], dt.bfloat16,
                             kind="ExternalInput").ap()
    wmat_d = nc.dram_tensor("wmat", [64, 2 * 64], dt.bfloat16,
                            kind="ExternalInput").ap()   # [d, (b,o)]
    lw_d = nc.dram_tensor("lw65", [65, 64], dt.bfloat16,
                          kind="ExternalInput").ap()
    iota_d = nc.dram_tensor("iota", [128, WIN], dt.bfloat16,
                            kind="ExternalInput").ap()
    out_d = nc.dram_tensor("out", [NPC, 64], dt.float32,
                           kind="ExternalOutput").ap()

    max_gchunks = int(max(GS)) // 128
    # max appearances in any pack (for tile sizing)
    pack_na = []
    for g in range(NG):
        for p in range(0, GW, PACK):
            ws = range(p, min(p + PACK, GW))
            a0 = app_ranges[(g, ws[0])][0]
            a1 = app_ranges[(g, ws[-1])][1]
            pack_na.append(a1 - a0)
    max_na = max(pack_na)

    with TileContext(nc) as tc:
        with (
            tc.tile_pool(name="const", bufs=1) as cpool,
            tc.tile_pool(name="gidx", bufs=2) as ipool,
            tc.tile_pool(name="gather", bufs=2) as gpool,
            tc.tile_pool(name="sel", bufs=2) as spool,
            tc.tile_pool(name="aggsb", bufs=2) as apool,
            tc.tile_pool(name="hout", bufs=2) as hpool,
            tc.tile_pool(name="psum_a", bufs=1, space="PSUM") as pa,
            tc.tile_pool(name="psum_h", bufs=2, space="PSUM") as ph,
        ):
            dstl_t = cpool.tile([128, A], dt.bfloat16)
            nc.scalar.dma_start(out=dstl_t[:], in_=dstl_d[:])
            cc_t = cpool.tile([128, 2 * A], dt.bfloat16)
            nc.scalar.dma_start(out=cc_t[:], in_=cc_d[:])
            featT_t = cpool.tile([65, NPC], dt.bfloat16)
            nc.scalar.dma_start(out=featT_t[:], in_=featT_d[:])
            wmat_t = cpool.tile([64, 2 * 64], dt.bfloat16)
            nc.scalar.dma_start(out=wmat_t[:], in_=wmat_d[:])
            lw_t = cpool.tile([65, 64], dt.bfloat16)
            nc.scalar.dma_start(out=lw_t[:], in_=lw_d[:])
            iota_t = cpool.tile([128, WIN], dt.bfloat16)
            nc.scalar.dma_start(out=iota_t[:], in_=iota_d[:])

            for g in range(NG):
                gsz = int(GS[g])
                git = ipool.tile([128, max_gchunks * 8], dt.int16, tag="i")
                nc.sync.dma_start(
                    out=git[:, : gsz // 16],
                    in_=idx_d[:, int(grp_off[g]) // 16:
                              (int(grp_off[g]) + gsz) // 16])
                gt = gpool.tile([128, max_gchunks, 128], dt.bfloat16, tag="g")
                for c in calls_by_g[g]:
                    i0 = (c["idx0"] - int(grp_off[g])) // 16
                    nch = c["nidx"] // 128
                    nc.gpsimd.dma_gather(
                        out_ap=gt[:, c["chunk0"]: c["chunk0"] + nch, :],
                        in_ap=table_d[c["bank"] * BANK:
                                      min((c["bank"] + 1) * BANK, N_NODES), :],
                        idxs_ap=git[:, i0: i0 + c["nidx"] // 16],
                        num_idxs=c["nidx"],
                        num_idxs_reg=c["nidx"],
                        elem_size=128,
                        queue_num=c["queue"],
                        single_packet=False,
                    )

                for p0 in range(0, GW, PACK):
                    if stage < 2:
                        break
                    ws = list(range(p0, min(p0 + PACK, GW)))
                    a0 = app_ranges[(g, ws[0])][0]
                    a1 = app_ranges[(g, ws[-1])][1]
                    na = a1 - a0
                    if na > 0:
                        oh = spool.tile([128, max_na, WIN], dt.bfloat16,
                                        tag="oh")
                        s01 = spool.tile([128, max_na, 2, WIN], dt.bfloat16,
                                         tag="s01")
                        nc.vector.tensor_tensor(
                            out=oh[:, :na, :],
                            in0=dstl_t[:, a0:a1].unsqueeze(-1)
                                .to_broadcast([128, na, WIN]),
                            in1=iota_t[:].unsqueeze(1)
                                .to_broadcast([128, na, WIN]),
                            op=mybir.AluOpType.is_equal,
                        )
                        nc.vector.tensor_tensor(
                            out=s01[:, :na, :, :],
                            in0=oh[:, :na, :].unsqueeze(2)
                                .to_broadcast([128, na, 2, WIN]),
                            in1=cc_t[:, 2 * a0: 2 * a1]
                                .rearrange("p (a c) -> p a c", c=2)
                                .unsqueeze(-1).to_broadcast([128, na, 2, WIN]),
                            op=mybir.AluOpType.mult,
                        )
                    aps = pa.tile([64, PACK * 2 * WIN], dt.float32,
                                  tag=f"a{p0 // PACK}", name=f"aps{p0}")
                    for wl in ws:
                        wa0, wa1 = app_ranges[(g, wl)]
                        col = (wl - p0) * 2 * WIN
                        for a in range(wa0, wa1):
                            ap_ = apps[a]
                            nc.tensor.matmul(
                                out=aps[:, col: col + 2 * WIN],
                                lhsT=gt[:, ap_["chunk"], 0:64],
                                rhs=s01[:, a - a0, :, :]
                                    .rearrange("p c w -> p (c w)"),
                                start=(a == wa0),
                                stop=(a == wa1 - 1),
                            )
                    aggs = apool.tile([64, PACK * 2 * WIN], dt.bfloat16,
                                      tag="aggs")
                    ncols = len(ws) * 2 * WIN
                    nc.scalar.activation(
                        out=aggs[:, :ncols], in_=aps[:, :ncols],
                        func=mybir.ActivationFunctionType.Copy)
                    hps = ph.tile([WIN, PACK, 64], dt.float32, tag="h")
                    for wl in ws:
                        wa0, wa1 = app_ranges[(g, wl)]
                        has = wa1 > wa0
                        col = (wl - p0) * 2 * WIN
                        hslice = hps[:, wl - p0, :]
                        if has:
                            for b2 in range(2):
                                nc.tensor.matmul(
                                    out=hslice,
                                    lhsT=aggs[:, col + b2 * WIN:
                                              col + (b2 + 1) * WIN],
                                    rhs=wmat_t[:, b2 * 64: b2 * 64 + 64],
                                    start=(b2 == 0),
                                    stop=False,
                                )
                        w_glob = g * GW + wl
                        nc.tensor.matmul(
                            out=hslice,
                            lhsT=featT_t[:, w_glob * WIN:
                                         w_glob * WIN + WIN],
                            rhs=lw_t[:],
                            start=(not has),
                            stop=True,
                        )
                    hs = hpool.tile([WIN, PACK, 64], dt.float32, tag="hs")
                    nc.scalar.activation(
                        out=hs[:, : len(ws), :], in_=hps[:, : len(ws), :],
                        func=mybir.ActivationFunctionType.Copy)
                    r0 = (g * GW + p0) * WIN
                    nc.sync.dma_start(
                        out=out_d[r0: r0 + len(ws) * WIN, :]
                            .rearrange("(w s) o -> s w o", w=len(ws)),
                        in_=hs[:, : len(ws), :])

    nc.compile()
    return nc


def make_inputs(sched, feat, W, coeff, h_bias, loop_weight):
    winf, slot = sched["winf"], sched["slot"]
    core_of, w_of = winf // NW, winf % NW

    table = np.zeros((N_NODES, 128), BF16)
    table[:, 0:64] = feat.astype(BF16)

    wmat = np.ascontiguousarray(
        W.transpose(1, 0, 2).reshape(64, 2 * 64)).astype(BF16)
    lw65 = np.concatenate(
        [loop_weight.astype(np.float32), h_bias[None].astype(np.float32)],
        0).astype(BF16)
    iota = np.tile(np.arange(WIN, dtype=np.float32)[None],
                   (128, 1)).astype(BF16)

    # idx wrap: [S_total] -> [128, S/16]
    idx = sched["idx_stream"]                     # [K, S_total] int16
    S = idx.shape[1]
    idxw = np.ascontiguousarray(
        idx.reshape(K, S // 16, 16).transpose(0, 2, 1))       # [K, 16, S/16]
    idxw = np.tile(idxw, (1, 8, 1))                           # [K, 128, S/16]

    dstl_app = sched["dstl_app"].astype(BF16)                 # [K, 128, A]
    cc_app = np.ascontiguousarray(
        sched["cc_app"].reshape(K, 128, -1)).astype(BF16)     # [K, 128, 2A]

    in_maps = []
    for k in range(K):
        fT = np.zeros((65, NPC), np.float32)
        fT[64, :] = 1.0
        mine = core_of == k
        rows = w_of[mine] * WIN + slot[mine]
        fT[0:64, rows] = feat[mine].T
        in_maps.append({
            "table": table,
            "idx": idxw[k],
            "dstl": dstl_app[k],
            "cc": cc_app[k],
            "featT": fT.astype(BF16),
            "wmat": wmat,
            "lw65": lw65,
            "iota": iota,
        })
    return in_maps


def run(feat, W, coeff, h_bias, loop_weight, src, dst, etypes,
        trace=False):
    sched = make_schedule(np.asarray(src, np.int64),
                          np.asarray(dst, np.int64),
                          np.asarray(etypes, np.int64),
                          np.asarray(coeff, np.float32))
    # annotate calls with bank (needed for in_ap slicing)
    for c in sched["calls"]:
        g = c["g"]
        # recover bank from chunk0 via seg_off
        so = sched["seg_off"][g] // 128
        b = int(np.searchsorted(so, c["chunk0"], side="right") - 1)
        c["bank"] = b
    nc = build_program(sched)
    in_maps = make_inputs(sched, np.asarray(feat, np.float32),
                          np.asarray(W, np.float32),
                          np.asarray(coeff, np.float32),
                          np.asarray(h_bias, np.float32),
                          np.asarray(loop_weight, np.float32))
    res = run_bass_kernel_spmd(nc, in_maps, list(range(K)), trace=trace)
    outs = np.stack([res.results[k]["out"] for k in range(K)])  # [K, NPC, 64]
    winf, slot = sched["winf"], sched["slot"]
    core_of, w_of = winf // NW, winf % NW
    h = outs[core_of, w_of * WIN + slot, :]
    return h.astype(np.float32), res


def kernel(feat, W, coeff, h_bias, loop_weight, src, dst, etypes):
    h, _ = run(feat, W, coeff, h_bias, loop_weight, src, dst, etypes)
    return h


# revision 5
# speedup vs baseline: 1.1312x; 1.1156x over previous
"""Trainium2 Bass kernel v2 for CuGraphRelGraphConv on 8 NeuronCores.

Dense-gather design (see sched2.py for the host schedule):
  - dst nodes snake-dealt by degree to 8 cores x 196 windows of 64 slots
    -> near-equal per-window edge counts across cores (SPMD-uniform).
  - per core: 14 groups x 14 windows; slot stream (group, bank, window, src)
    dense, segment sizes padded to cross-core max (128-aligned, pad idx 0).
  - gather: per-(g,b) segments split into <=2048-idx sub-calls round-robin
    over all 4 SWDGE queues -> ~1.5-2.0 ns/idx sustained.
  - scatter: per "appearance" (tile x window) one 128-col matmul
    G_chunk^T @ S01 accumulating into a 4-window PSUM pack [64, 512];
    S01 built on DVE from per-appearance dstl/cc metadata (sentinel 64
    for foreign slots of straddle tiles).
  - h = sum_b aggT_b @ W_b + [feat|1] @ [loop_w; bias] per window,
    copied out via per-pack DMA.
"""
import sys

sys.path.insert(0, "/opt/trn_rl_repo")

import numpy as np
import ml_dtypes

import concourse.bass as bass
import concourse.bacc as bacc
import concourse.mybir as mybir
from concourse.bass_utils import run_bass_kernel_spmd
from concourse.tile import TileContext

import numpy as np
N_NODES = 100000
N_EDGES = 1600000
K = 8
WIN = 32
NG = 14           # groups per core
_wins_per_core = -(-(-(-N_NODES // WIN) // K))
GW = -(-_wins_per_core // NG)   # windows per group
NW = NG * GW      # windows per core
NB = 4            # src banks (int16 gather reach)
BANK = 32768
NPC = NW * WIN
SUBCALL = 1920   # 121 descriptors < 128-deep SWDGE ring
SENT = float(WIN)  # dstl sentinel


def assign_nodes(dst, src):
    """Deal nodes by degree to (core, window, slot); within each round's
    window position, permute the 8 candidate nodes across cores to balance
    cumulative per-(window, src-bank) edge counts (cuts cross-core pad)."""
    deg = np.bincount(dst, minlength=N_NODES)
    degb = np.zeros((N_NODES, NB), np.int64)
    np.add.at(degb, (dst, src >> 15), 1)
    order = np.argsort(-deg, kind="stable")
    nwt = K * NW
    winf = np.empty(N_NODES, np.int64)
    slot = np.empty(N_NODES, np.int64)
    cum = np.zeros((K, NW, NB), np.float64)
    r = 0
    for off in range(0, N_NODES, nwt):
        ch = order[off: off + nwt]
        # nodes_rw[w, k] = node that plain snake would put at col k*NW+w
        nodes_rw = np.full((NW, K), -1, np.int64)
        cols = np.arange(len(ch))
        if r % 2 == 1:
            cols = nwt - 1 - cols
        nodes_rw[cols % NW, cols // NW] = ch
        # greedy per window position: biggest node first, to the core
        # minimizing sum_b (cum + deg)^2
        ordk = np.argsort(-deg[np.maximum(nodes_rw, 0)]
                          - 10**9 * (nodes_rw < 0), axis=1)   # [NW, K]
        used = np.zeros((NW, K), bool)
        rows = np.arange(NW)
        for step in range(K):
            v = nodes_rw[rows, ordk[:, step]]                 # [NW]
            ok = v >= 0
            d = degb[np.maximum(v, 0)]                        # [NW, NB]
            cost = ((cum.transpose(1, 0, 2) + d[:, None, :]) ** 2
                    ).sum(axis=2)                             # [NW, K]
            cost[used] = np.inf
            kk = np.argmin(cost, axis=1)                      # [NW]
            used[rows, kk] = True
            vv, kks, ws = v[ok], kk[ok], rows[ok]
            winf[vv] = kks * NW + ws
            slot[vv] = r
            cum[kks, ws] += degb[vv]
        r += 1
    assert slot.max() < WIN
    return winf, slot


def make_schedule(src, dst, etypes, coeff):
    src = np.asarray(src, np.int64)
    dst = np.asarray(dst, np.int64)
    etypes = np.asarray(etypes, np.int64)
    coeff = np.asarray(coeff, np.float32)

    winf, slot = assign_nodes(dst, src)
    core_of, w_of = winf // NW, winf % NW

    ek = core_of[dst]                  # core
    ew = w_of[dst]                     # window in core
    eg = ew // GW                      # group
    ewl = ew % GW                      # window in group
    eb = src >> 15                     # bank
    eidx = (src & (BANK - 1)).astype(np.int64)
    edstl = slot[dst]                  # 0..63
    ecc = coeff[etypes]                # [E, 2]

    # counts per (k, g, b, wl)
    key = ((ek * NG + eg) * NB + eb) * GW + ewl
    C = np.bincount(key, minlength=K * NG * NB * GW) \
        .reshape(K, NG, NB, GW).astype(np.int64)
    S = C.sum(axis=3)                                  # [K, NG, NB]
    Sstar = 128 * np.ceil(S.max(axis=0) / 128).astype(np.int64)  # [NG, NB]

    starts = np.cumsum(C, axis=3) - C                  # excl cumsum [K,NG,NB,GW]
    ends = starts + C
    lo = starts.min(axis=0)                            # [NG, NB, GW]
    hi = ends.max(axis=0)

    seg_off = np.cumsum(Sstar, axis=1) - Sstar         # [NG, NB] within group
    GS = Sstar.sum(axis=1)                             # group sizes
    grp_off = np.cumsum(GS) - GS
    S_total = int(GS.sum())

    # --- edge slot positions ---
    # order edges by (k, g, b, wl, src); per-bucket running position
    perm = np.lexsort((src, ewl, eb, eg, ek))
    k_s, g_s, b_s, wl_s = ek[perm], eg[perm], eb[perm], ewl[perm]
    bucket = ((k_s * NG + g_s) * NB + b_s) * GW + wl_s
    change = np.r_[True, bucket[1:] != bucket[:-1]]
    run_start = np.flatnonzero(change)
    run_id = np.cumsum(change) - 1
    pos = np.arange(len(perm)) - run_start[run_id]
    spos = (grp_off[g_s] + seg_off[g_s, b_s]
            + starts[k_s, g_s, b_s, wl_s] + pos)       # slot within core

    # per-core slot arrays
    idx_stream = np.zeros((K, S_total), np.int16)
    wl_slot = np.full((K, S_total), -1, np.int64)
    dstl_slot = np.full((K, S_total), WIN, np.float32)
    cc_slot = np.zeros((K, S_total, 2), np.float32)
    idx_stream[k_s, spos] = eidx[perm].astype(np.int16)
    wl_slot[k_s, spos] = wl_s
    dstl_slot[k_s, spos] = edstl[perm]
    cc_slot[k_s, spos] = ecc[perm]

    # --- appearances ---
    # per (g, w): ordered list over b, tiles t in seg (g,b) where
    # [lo, hi) of (g,b,w) intersects tile t. Order: (g, w, b, t).
    apps = []            # dicts: g, wl, b, t, chunk (within group), a (index)
    app_ranges = {}      # (g, wl) -> (a0, a1) contiguous? order by (g,w)
    for g in range(NG):
        for wl in range(GW):
            first = len(apps)
            for b in range(NB):
                l, h = lo[g, b, wl], hi[g, b, wl]
                if h <= l:
                    continue
                t0, t1 = l // 128, (h - 1) // 128 + 1
                for t in range(t0, t1):
                    apps.append(dict(
                        g=g, wl=wl, b=b, t=t,
                        chunk=(seg_off[g, b] // 128) + t))
            app_ranges[(g, wl)] = (first, len(apps))
    A = len(apps)

    # appearance metadata arrays [K, 128, A]
    dstl_app = np.full((K, 128, A), SENT, np.float32)
    cc_app = np.zeros((K, 128, A, 2), np.float32)
    for a, ap in enumerate(apps):
        g, wl, b, t = ap["g"], ap["wl"], ap["b"], ap["t"]
        s0 = grp_off[g] + seg_off[g, b] + 128 * t
        sl = slice(s0, s0 + 128)
        m = wl_slot[:, sl] == wl                       # [K, 128]
        dstl_app[:, :, a] = np.where(m, dstl_slot[:, sl], SENT)
        cc_app[:, :, a, :] = cc_slot[:, sl, :]

    # --- gather calls ---
    calls = []   # (g, col0, ncols16, chunk0, nchunks, queue)
    q = 0
    for g in range(NG):
        for b in range(NB):
            sz = int(Sstar[g, b])
            off = 0
            while off < sz:
                nn = min(SUBCALL, sz - off)
                s0 = grp_off[g] + seg_off[g, b] + off
                calls.append(dict(
                    g=g, idx0=int(s0), nidx=int(nn),
                    chunk0=int((seg_off[g, b] + off) // 128),
                    queue=q % 4))
                q += 1
                off += nn

    return dict(
        winf=winf, slot=slot, C=C, Sstar=Sstar, seg_off=seg_off, GS=GS,
        grp_off=grp_off, S_total=S_total, idx_stream=idx_stream,
        apps=apps, app_ranges=app_ranges, A=A,
        dstl_app=dstl_app, cc_app=cc_app, calls=calls,
        wl_slot=wl_slot,
    )



BF16 = ml_dtypes.bfloat16
PACK = 512 // (2 * WIN)       # windows per PSUM pack ([64, 512] fp32 bank)


def build_program(sched):
    import os
    stage = int(os.environ.get("K2_STAGE", "9"))
    dt = mybir.dt
    GS = sched["GS"]
    grp_off = sched["grp_off"]
    S_total = sched["S_total"]
    A = sched["A"]
    apps = sched["apps"]
    app_ranges = sched["app_ranges"]
    calls = sched["calls"]
    seg_off = sched["seg_off"]

    calls_by_g = [[c for c in calls if c["g"] == g] for g in range(NG)]

    nc = bacc.Bacc("TRN2", target_bir_lowering=False, debug=False,
                   num_devices=K, num_swdge_queues=4)

    table_d = nc.dram_tensor("table", [N_NODES, 128], dt.bfloat16,
                             kind="ExternalInput").ap()
    idx_d = nc.dram_tensor("idx", [128, S_total // 16], dt.int16,
                           kind="ExternalInput").ap()
    dstl_d = nc.dram_tensor("dstl", [128, A], dt.bfloat16,
                            kind="ExternalInput").ap()
    cc_d = nc.dram_tensor("cc", [128, 2 * A], dt.bfloat16,
                          kind="ExternalInput").ap()
    featT_d = nc.dram_tensor("featT", [65, NPC], dt.bfloat16,
                             kind="ExternalInput").ap()
    wmat_d = nc.dram_tensor("wmat", [64, 2 * 64], dt.bfloat16,
                            kind="ExternalInput").ap()   # [d, (b,o)]
    lw_d = nc.dram_tensor("lw65", [65, 64], dt.bfloat16,
                          kind="ExternalInput").ap()
    iota_d = nc.dram_tensor("iota", [128, WIN], dt.bfloat16,
                            kind="ExternalInput").ap()
    out_d = nc.dram_tensor("out", [NPC, 64], dt.float32,
                           kind="ExternalOutput").ap()

    max_gchunks = int(max(GS)) // 128
    # max appearances in any pack (for tile sizing)
    pack_na = []
    for g in range(NG):
        for p in range(0, GW, PACK):
            ws = range(p, min(p + PACK, GW))
            a0 = app_ranges[(g, ws[0])][0]
            a1 = app_ranges[(g, ws[-1])][1]
            pack_na.append(a1 - a0)
    max_na = max(pack_na)

    with TileContext(nc) as tc:
        with (
            tc.tile_pool(name="const", bufs=1) as cpool,
            tc.tile_pool(name="gidx", bufs=2) as ipool,
            tc.tile_pool(name="gather", bufs=2) as gpool,
            tc.tile_pool(name="sel", bufs=2) as spool,
            tc.tile_pool(name="aggsb", bufs=2) as apool,
            tc.tile_pool(name="hout", bufs=2) as hpool,
            tc.tile_pool(name="psum_a", bufs=1, space="PSUM") as pa,
            tc.tile_pool(name="psum_h", bufs=2, space="PSUM") as ph,
        ):
            dstl_t = cpool.tile([128, A], dt.bfloat16)
            nc.scalar.dma_start(out=dstl_t[:], in_=dstl_d[:])
            cc_t = cpool.tile([128, 2 * A], dt.bfloat16)
            nc.scalar.dma_start(out=cc_t[:], in_=cc_d[:])
            featT_t = cpool.tile([65, NPC], dt.bfloat16)
            nc.scalar.dma_start(out=featT_t[:], in_=featT_d[:])
            wmat_t = cpool.tile([64, 2 * 64], dt.bfloat16)
            nc.scalar.dma_start(out=wmat_t[:], in_=wmat_d[:])
            lw_t = cpool.tile([65, 64], dt.bfloat16)
            nc.scalar.dma_start(out=lw_t[:], in_=lw_d[:])
            iota_t = cpool.tile([128, WIN], dt.bfloat16)
            nc.scalar.dma_start(out=iota_t[:], in_=iota_d[:])

            for g in range(NG):
                gsz = int(GS[g])
                git = ipool.tile([128, max_gchunks * 8], dt.int16, tag="i")
                nc.sync.dma_start(
                    out=git[:, : gsz // 16],
                    in_=idx_d[:, int(grp_off[g]) // 16:
                              (int(grp_off[g]) + gsz) // 16])
                gt = gpool.tile([128, max_gchunks, 128], dt.bfloat16, tag="g")
                for c in calls_by_g[g]:
                    i0 = (c["idx0"] - int(grp_off[g])) // 16
                    nch = c["nidx"] // 128
                    nc.gpsimd.dma_gather(
                        out_ap=gt[:, c["chunk0"]: c["chunk0"] + nch, :],
                        in_ap=table_d[c["bank"] * BANK:
                                      min((c["bank"] + 1) * BANK, N_NODES), :],
                        idxs_ap=git[:, i0: i0 + c["nidx"] // 16],
                        num_idxs=c["nidx"],
                        num_idxs_reg=c["nidx"],
                        elem_size=128,
                        queue_num=c["queue"],
                        single_packet=False,
                    )

                for p0 in range(0, GW, PACK):
                    if stage < 2:
                        break
                    ws = list(range(p0, min(p0 + PACK, GW)))
                    a0 = app_ranges[(g, ws[0])][0]
                    a1 = app_ranges[(g, ws[-1])][1]
                    na = a1 - a0
                    if na > 0:
                        oh = spool.tile([128, max_na, WIN], dt.bfloat16,
                                        tag="oh")
                        s01 = spool.tile([128, max_na, 2, WIN], dt.bfloat16,
                                         tag="s01")
                        nc.vector.tensor_tensor(
                            out=oh[:, :na, :],
                            in0=dstl_t[:, a0:a1].unsqueeze(-1)
                                .to_broadcast([128, na, WIN]),
                            in1=iota_t[:].unsqueeze(1)
                                .to_broadcast([128, na, WIN]),
                            op=mybir.AluOpType.is_equal,
                        )
                        nc.vector.tensor_tensor(
                            out=s01[:, :na, :, :],
                            in0=oh[:, :na, :].unsqueeze(2)
                                .to_broadcast([128, na, 2, WIN]),
                            in1=cc_t[:, 2 * a0: 2 * a1]
                                .rearrange("p (a c) -> p a c", c=2)
                                .unsqueeze(-1).to_broadcast([128, na, 2, WIN]),
                            op=mybir.AluOpType.mult,
                        )
                    aps = pa.tile([64, PACK * 2 * WIN], dt.float32,
                                  tag=f"a{p0 // PACK}", name=f"aps{p0}")
                    for wl in ws:
                        wa0, wa1 = app_ranges[(g, wl)]
                        col = (wl - p0) * 2 * WIN
                        for a in range(wa0, wa1):
                            ap_ = apps[a]
                            nc.tensor.matmul(
                                out=aps[:, col: col + 2 * WIN],
                                lhsT=gt[:, ap_["chunk"], 0:64],
                                rhs=s01[:, a - a0, :, :]
                                    .rearrange("p c w -> p (c w)"),
                                start=(a == wa0),
                                stop=(a == wa1 - 1),
                            )
                    aggs = apool.tile([64, PACK * 2 * WIN], dt.bfloat16,
                                      tag="aggs")
                    ncols = len(ws) * 2 * WIN
                    nc.scalar.activation(
                        out=aggs[:, :ncols], in_=aps[:, :ncols],
                        func=mybir.ActivationFunctionType.Copy)
                    hps = ph.tile([WIN, PACK, 64], dt.float32, tag="h")
                    for wl in ws:
                        wa0, wa1 = app_ranges[(g, wl)]
                        has = wa1 > wa0
                        col = (wl - p0) * 2 * WIN
                        hslice = hps[:, wl - p0, :]
                        if has:
                            for b2 in range(2):
                                nc.tensor.matmul(
                                    out=hslice,
                                    lhsT=aggs[:, col + b2 * WIN:
                                              col + (b2 + 1) * WIN],
                                    rhs=wmat_t[:, b2 * 64: b2 * 64 + 64],
                                    start=(b2 == 0),
                                    stop=False,
                                )
                        w_glob = g * GW + wl
                        nc.tensor.matmul(
                            out=hslice,
                            lhsT=featT_t[:, w_glob * WIN:
                                         w_glob * WIN + WIN],
                            rhs=lw_t[:],
                            start=(not has),
                            stop=True,
                        )
                    hs = hpool.tile([WIN, PACK, 64], dt.float32, tag="hs")
                    nc.scalar.activation(
                        out=hs[:, : len(ws), :], in_=hps[:, : len(ws), :],
                        func=mybir.ActivationFunctionType.Copy)
                    r0 = (g * GW + p0) * WIN
                    nc.scalar.dma_start(
                        out=out_d[r0: r0 + len(ws) * WIN, :]
                            .rearrange("(w s) o -> s w o", w=len(ws)),
                        in_=hs[:, : len(ws), :])

    nc.compile()
    return nc


def make_inputs(sched, feat, W, coeff, h_bias, loop_weight):
    winf, slot = sched["winf"], sched["slot"]
    core_of, w_of = winf // NW, winf % NW

    table = np.zeros((N_NODES, 128), BF16)
    table[:, 0:64] = feat.astype(BF16)

    wmat = np.ascontiguousarray(
        W.transpose(1, 0, 2).reshape(64, 2 * 64)).astype(BF16)
    lw65 = np.concatenate(
        [loop_weight.astype(np.float32), h_bias[None].astype(np.float32)],
        0).astype(BF16)
    iota = np.tile(np.arange(WIN, dtype=np.float32)[None],
                   (128, 1)).astype(BF16)

    # idx wrap: [S_total] -> [128, S/16]
    idx = sched["idx_stream"]                     # [K, S_total] int16
    S = idx.shape[1]
    idxw = np.ascontiguousarray(
        idx.reshape(K, S // 16, 16).transpose(0, 2, 1))       # [K, 16, S/16]
    idxw = np.tile(idxw, (1, 8, 1))                           # [K, 128, S/16]

    dstl_app = sched["dstl_app"].astype(BF16)                 # [K, 128, A]
    cc_app = np.ascontiguousarray(
        sched["cc_app"].reshape(K, 128, -1)).astype(BF16)     # [K, 128, 2A]

    in_maps = []
    for k in range(K):
        fT = np.zeros((65, NPC), np.float32)
        fT[64, :] = 1.0
        mine = core_of == k
        rows = w_of[mine] * WIN + slot[mine]
        fT[0:64, rows] = feat[mine].T
        in_maps.append({
            "table": table,
            "idx": idxw[k],
            "dstl": dstl_app[k],
            "cc": cc_app[k],
            "featT": fT.astype(BF16),
            "wmat": wmat,
            "lw65": lw65,
            "iota": iota,
        })
    return in_maps


def run(feat, W, coeff, h_bias, loop_weight, src, dst, etypes,
        trace=False):
    sched = make_schedule(np.asarray(src, np.int64),
                          np.asarray(dst, np.int64),
                          np.asarray(etypes, np.int64),
                          np.asarray(coeff, np.float32))
    # annotate calls with bank (needed for in_ap slicing)
    for c in sched["calls"]:
        g = c["g"]
        # recover bank from chunk0 via seg_off
        so = sched["seg_off"][g] // 128
        b = int(np.searchsorted(so, c["chunk0"], side="right") - 1)
        c["bank"] = b
    nc = build_program(sched)
    in_maps = make_inputs(sched, np.asarray(feat, np.float32),
                          np.asarray(W, np.float32),
                          np.asarray(coeff, np.float32),
                          np.asarray(h_bias, np.float32),
                          np.asarray(loop_weight, np.float32))
    res = run_bass_kernel_spmd(nc, in_maps, list(range(K)), trace=trace)
    outs = np.stack([res.results[k]["out"] for k in range(K)])  # [K, NPC, 64]
    winf, slot = sched["winf"], sched["slot"]
    core_of, w_of = winf // NW, winf % NW
    h = outs[core_of, w_of * WIN + slot, :]
    return h.astype(np.float32), res


def kernel(feat, W, coeff, h_bias, loop_weight, src, dst, etypes):
    h, _ = run(feat, W, coeff, h_bias, loop_weight, src, dst, etypes)
    return h
